# revision 1
# baseline (speedup 1.0000x reference)
"""Bass/Trainium2 kernel for nn_Attention_21354577395789.

Reference computation (B=16, S=2048, H=1024, D=2H=2048):
    h      = broadcast(hidden[1, 2H]) -> [B, S, 2H]
    cat    = concat([h, enc], -1)                    [B, S, 4H]
    energy = tanh(cat @ attn_w.T + attn_b)           [B, S, H]
    scores = energy @ v_w.T                          [B, S, 1]
    attn   = softmax(scores, axis=1)
    ctx    = attn^T @ enc                            [B, 1, 2H]

Key algebraic simplification: split attn_w = [W_h | W_e] along its 4H input
dim. Then  cat @ attn_w.T = hidden @ W_h.T + enc @ W_e.T, and
c = hidden @ W_h.T + attn_b is a single [H] vector shared by every (b, s).
This halves the FLOPs; the surviving big matmul is enc @ W_e.T.

Distribution: pure data-parallel over B across 8 NeuronCores (2 batches per
core), no collectives. Compute in bf16 (fp32 PSUM accumulation).
"""

import os

import numpy as np
import ml_dtypes

B, S, H = 16, 2048, 1024
D = 2 * H          # 2048, encoder feature dim / contraction dim of W_e
N_CORES = 8
BPC = B // N_CORES  # batches per core = 2
NT = 512           # t-block (moving-dim) size

BF16 = ml_dtypes.bfloat16

_cache = {}


def _build(BPC=BPC, S=S, H=H, D=D, NT=NT):
    KT = D // 128      # k-tiles of 128 over the contraction dim d
    JT = H // 128      # j-tiles of 128 over the energy dim
    TBLK = S // NT     # t-blocks per batch
    DBLK = D // NT     # d-blocks per batch (context)
    TT = S // 128      # t-tiles of 128 (context contraction)
    import concourse.bacc as bacc
    import concourse.tile as tile
    from concourse import mybir

    nc = bacc.Bacc("TRN2", target_bir_lowering=False, debug=False)
    dt = mybir.dt

    encT = nc.declare_dram_parameter("encT", [BPC, D, S], dt.bfloat16, isOutput=False)
    encN = nc.declare_dram_parameter("encN", [BPC, S, D], dt.bfloat16, isOutput=False)
    w_eT = nc.declare_dram_parameter("w_eT", [D, H], dt.bfloat16, isOutput=False)
    w_hT = nc.declare_dram_parameter("w_hT", [D, H], dt.bfloat16, isOutput=False)
    h_cols = nc.declare_dram_parameter("h_cols", [128, KT], dt.bfloat16, isOutput=False)
    v_cols = nc.declare_dram_parameter("v_cols", [128, JT], dt.bfloat16, isOutput=False)
    b_row = nc.declare_dram_parameter("b_row", [1, H], dt.float32, isOutput=False)
    out = nc.declare_dram_parameter("out", [BPC, D], dt.float32, isOutput=True)

    AF = mybir.ActivationFunctionType
    AX = mybir.AxisListType

    with tile.TileContext(nc) as tc:
        with (
            tc.tile_pool(name="weights", bufs=1) as wpool,
            tc.tile_pool(name="whstream", bufs=2) as whpool,
            tc.tile_pool(name="enct", bufs=2) as enct_pool,
            tc.tile_pool(name="encn", bufs=2) as encn_pool,
            tc.tile_pool(name="energy", bufs=6) as epool,
            tc.tile_pool(name="small", bufs=4) as spool,
            tc.tile_pool(name="perb", bufs=2) as bpool,
            tc.tile_pool(name="psum_e", bufs=5, space="PSUM") as pe_pool,
            tc.tile_pool(name="psum_s", bufs=2, space="PSUM") as ps_pool,
        ):
            # ---- resident weights / constants -------------------------
            hT_sb = wpool.tile([128, KT], dt.bfloat16, tag="hT")
            nc.sync.dma_start(hT_sb[:], h_cols.ap()[:])
            v_sb = wpool.tile([128, JT], dt.bfloat16, tag="v")
            nc.sync.dma_start(v_sb[:], v_cols.ap()[:])
            brow_sb = wpool.tile([1, H], dt.float32, tag="brow")
            nc.sync.dma_start(brow_sb[:], b_row.ap()[:])

            # ---- c = hidden @ W_h.T + attn_b  ([1, H] then -> [128, JT])
            c_row = wpool.tile([1, H], dt.float32, tag="crow")
            halves = [(h0, min(512, H - h0)) for h0 in range(0, H, 512)]
            c_ps = {}
            for h0, hw in halves:
                c_ps[h0] = ps_pool.tile(
                    [1, hw], dt.float32, tag="sps", name=f"cps{h0}"
                )
            for kk in range(KT):
                wh_t = whpool.tile([128, H], dt.bfloat16, tag="whs")
                nc.sync.dma_start(
                    wh_t[:], w_hT.ap()[kk * 128 : (kk + 1) * 128, :]
                )
                for h0, hw in halves:
                    nc.tensor.matmul(
                        c_ps[h0][:],
                        hT_sb[:, kk : kk + 1],
                        wh_t[:, h0 : h0 + hw],
                        start=(kk == 0),
                        stop=(kk == KT - 1),
                    )
            for h0, hw in halves:
                nc.vector.tensor_add(
                    c_row[0:1, h0 : h0 + hw],
                    c_ps[h0][:],
                    brow_sb[0:1, h0 : h0 + hw],
                )
            w_eT_sb = wpool.tile([128, KT * H], dt.bfloat16, tag="weT")   # blk kk: [128d, H]
            for kk in range(KT):
                nc.sync.dma_start(
                    w_eT_sb[:, kk * H : (kk + 1) * H],
                    w_eT.ap()[kk * 128 : (kk + 1) * 128, :],
                )
            c_cols = wpool.tile([128, JT], dt.float32, tag="ccols")
            for jj in range(JT):
                nc.sync.dma_start(
                    c_cols[:, jj : jj + 1],
                    c_row[0:1, jj * 128 : (jj + 1) * 128],
                )

            # ---- phases: energy/scores/softmax per batch, with the
            # previous batch's context blocks interleaved between energy
            # groups so ctx matmuls + encN DMAs hide inside energy compute.
            TG = 2                       # t-blocks per energy group
            NG = TBLK // TG              # energy groups per batch
            DPG = DBLK // NG             # ctx d-blocks interleaved per group
            scores_sb = []
            w_cols_b = []
            rs_b = []
            out_rows = {}

            def ctx_block(cb, db):
                if db == 0:
                    out_rows[cb] = bpool.tile(
                        [1, D], dt.float32, tag="outrow", name=f"outrow{cb}"
                    )
                encn_t = encn_pool.tile([128, TT * NT], dt.bfloat16, tag="encn")
                for tt in range(TT):
                    nc.sync.dma_start(
                        encn_t[:, tt * NT : (tt + 1) * NT],
                        encN.ap()[
                            cb, tt * 128 : (tt + 1) * 128,
                            db * NT : (db + 1) * NT,
                        ],
                    )
                x_ps = ps_pool.tile([1, NT], dt.float32, tag="xps", bufs=1)
                for tt in range(TT):
                    nc.tensor.matmul(
                        x_ps[:],
                        w_cols_b[cb][:, tt : tt + 1],
                        encn_t[:, tt * NT : (tt + 1) * NT],
                        start=(tt == 0),
                        stop=(tt == TT - 1),
                    )
                nc.vector.tensor_scalar_mul(
                    out_rows[cb][0:1, db * NT : (db + 1) * NT], x_ps[:], rs_b[cb][:]
                )
                if db == DBLK - 1:
                    nc.sync.dma_start(out.ap()[cb : cb + 1, :], out_rows[cb][:])

            for b in range(BPC):
                sc = bpool.tile([1, S], dt.float32, tag="scores")
                scores_sb.append(sc)
                GW = TG * NT  # columns per group block
                for g in range(NG):
                    tbs = [g * TG + i for i in range(TG)]
                    enct_t = enct_pool.tile([128, KT * GW], dt.bfloat16, tag="enct")
                    for kk in range(KT):
                        nc.sync.dma_start(
                            enct_t[:, kk * GW : (kk + 1) * GW],
                            encT.ap()[
                                b, kk * 128 : (kk + 1) * 128,
                                g * GW : (g + 1) * GW,
                            ],
                        )
                    s_ps = {}
                    for tb in tbs:
                        s_ps[tb] = ps_pool.tile(
                            [1, NT], dt.float32, tag="sps", name=f"sps{tb}"
                        )
                    # software-pipeline the v-reduction one jj behind the
                    # energy matmuls so PE never waits on ACT's tanh
                    pending = []  # [(e_sb, jj, tb)]
                    for jj in range(JT):
                        e_ps = {}
                        for tb in tbs:
                            e_ps[tb] = pe_pool.tile(
                                [128, NT], dt.float32, tag="eps", name=f"eps{tb}"
                            )
                        for kk in range(KT):
                            w_ap = w_eT_sb[
                                :, kk * H + jj * 128 : kk * H + jj * 128 + 128
                            ]
                            for tb in tbs:
                                nc.tensor.matmul(
                                    e_ps[tb][:],
                                    w_ap,
                                    enct_t[
                                        :,
                                        kk * GW + (tb - g * TG) * NT
                                        : kk * GW + (tb - g * TG + 1) * NT,
                                    ],
                                    start=(kk == 0),
                                    stop=(kk == KT - 1),
                                )
                        flush = pending
                        pending = []
                        for tb in tbs:
                            e_sb = epool.tile([128, NT], dt.bfloat16, tag="energy")
                            nc.scalar.activation(
                                e_sb[:], e_ps[tb][:], AF.Tanh,
                                bias=c_cols[:, jj : jj + 1],
                            )
                            pending.append((e_sb, jj, tb))
                        for pe_sb, pjj, ptb in flush:
                            nc.tensor.matmul(
                                s_ps[ptb][:],
                                v_sb[:, pjj : pjj + 1],
                                pe_sb[:],
                                start=(pjj == 0),
                                stop=False,
                            )
                    for pe_sb, pjj, ptb in pending:
                        nc.tensor.matmul(
                            s_ps[ptb][:], v_sb[:, pjj : pjj + 1], pe_sb[:],
                            start=False, stop=True,
                        )
                    for tb in tbs:
                        nc.vector.tensor_copy(
                            sc[0:1, tb * NT : (tb + 1) * NT], s_ps[tb][:]
                        )
                    if b >= 1:
                        for i in range(DPG):
                            ctx_block(b - 1, g * DPG + i)
                # softmax over S (1 partition, free axis)
                mx = spool.tile([1, 1], dt.float32, tag="mx")
                nc.vector.reduce_max(mx[:], sc[:], axis=AX.X)
                nmx = spool.tile([1, 1], dt.float32, tag="nmx")
                nc.scalar.mul(nmx[:], mx[:], -1.0)
                w_row = bpool.tile([1, S], dt.bfloat16, tag="wrow")
                ssum = spool.tile([1, 1], dt.float32, tag="ssum")
                nc.scalar.activation(
                    w_row[:], sc[:], AF.Exp, bias=nmx[:], accum_out=ssum[:]
                )
                rs = spool.tile([1, 1], dt.float32, tag="rs")
                nc.vector.reciprocal(rs[:], ssum[:])
                rs_b.append(rs)
                w_cols = bpool.tile([128, TT], dt.bfloat16, tag="wcols")
                for tt in range(TT):
                    nc.sync.dma_start(
                        w_cols[:, tt : tt + 1],
                        w_row[0:1, tt * 128 : (tt + 1) * 128],
                    )
                w_cols_b.append(w_cols)

            # trailing context for the last batch
            for db in range(DBLK):
                ctx_block(BPC - 1, db)

    nc.compile()
    return nc


def _get_nc():
    if "nc" not in _cache:
        import time

        t0 = time.time()
        _cache["nc"] = _build()
        if os.environ.get("KERNEL_TRACE"):
            print(f"[kernel] bass build+compile: {time.time() - t0:.1f} s")
    return _cache["nc"]


def kernel(hidden, encoder_outputs, attn_w, attn_b, v_w):
    from concourse.bass_utils import run_bass_kernel_spmd

    nc = _get_nc()

    hidden = np.asarray(hidden, dtype=np.float32)
    enc = np.asarray(encoder_outputs, dtype=np.float32)
    attn_w = np.asarray(attn_w, dtype=np.float32)
    attn_b = np.asarray(attn_b, dtype=np.float32)
    v_w = np.asarray(v_w, dtype=np.float32)

    w_eT = np.ascontiguousarray(attn_w[:, D:].T).astype(BF16)   # [D, H]
    w_hT = np.ascontiguousarray(attn_w[:, :D].T).astype(BF16)   # [D, H]
    h_cols = np.ascontiguousarray(hidden.reshape(D // 128, 128).T).astype(BF16)
    v_cols = np.ascontiguousarray(v_w.reshape(H // 128, 128).T).astype(BF16)
    b_row = attn_b.reshape(1, H)

    in_maps = []
    for c in range(N_CORES):
        sl = enc[c * BPC : (c + 1) * BPC]
        in_maps.append(
            {
                "encT": np.swapaxes(sl, 1, 2).astype(BF16),
                "encN": sl.astype(BF16),
                "w_eT": w_eT,
                "w_hT": w_hT,
                "h_cols": h_cols,
                "v_cols": v_cols,
                "b_row": b_row,
            }
        )

    trace = bool(os.environ.get("KERNEL_TRACE"))
    if trace:
        _install_prof_shim()
    res = run_bass_kernel_spmd(
        nc, in_maps, core_ids=list(range(N_CORES)), trace=trace
    )
    if trace:
        _cache["last_exec_time_ns"] = res.exec_time_ns
        print(f"HW exec time: {res.exec_time_ns} ns")

    ctx = np.concatenate([res.results[c]["out"] for c in range(N_CORES)], axis=0)
    return ctx.reshape(B, 1, D).astype(np.float32)


def _install_prof_shim():
    """antenv.axon_hooks is absent from this image; inject it so
    run_bass_kernel_spmd(trace=True) can capture NTFF profiles."""
    import sys
    import types

    if "antenv.axon_hooks" in sys.modules:
        return
    import antenv

    mod = types.ModuleType("antenv.axon_hooks")
    mod._hook = None
    mod.set_axon_ntff_profile_hook = lambda h: setattr(mod, "_hook", h)
    mod.get_axon_ntff_profile_hook = lambda: mod._hook
    sys.modules["antenv.axon_hooks"] = mod
    antenv.axon_hooks = mod
    try:
        from trn_agent_boot.trn_boot import _ntff_profile_via_ctypes

        mod.set_axon_ntff_profile_hook(
            _ntff_profile_via_ctypes("/opt/axon/libaxon_pjrt.so")
        )
    except Exception:
        pass

